# revision 10
# baseline (speedup 1.0000x reference)
"""Trainium2 Bass kernel for DiagonalGMMPosterior (vq_codebook).

Reference computation (per batch b, descriptor n, cluster k):
    dist[k,n]  = sum_d (x[d,n] - mu_n[k,d])^2 * exp(-log_sigma[k,d])
    logits     = -dist + log_alpha[k] - 0.5 * sum_d log_sigma[k,d]
    out[k,n]   = softmax_k(logits)

Device strategy (8 NeuronCores, data-parallel over the batch axis):
  * Host folds all (K,D) parameter math into two GEMM weight matrices and
    a per-cluster constant, then CENTERS them across K (softmax is
    invariant to per-n shifts) so logits stay within ~±17 — no per-n max.
  * x is pre-converted to fp16 on host: halves input HBM traffic.
  * Two 1024-column halves A/B of each 2048-column group are STACKED on
    the 128 PSUM partitions (PE column tiling): A's dist GEMMs write
    partitions 0:64, B's write 64:128.  Everything downstream processes
    both halves in one 128-partition pass.
  * Pipeline per group (PW=1024 stacked cols):
       xsq  = x*x                  split DVE (fp16 2x mode) / ScalarE / Pool
       pd   = W1^T x^2 + W2^T x    TensorE fp16, A/B col-tiles interleaved
       et   = exp(pd + cc) -> bf16 ScalarE (bf16 holds e^±17; lets the
                                   ones-GEMM stream 2.5x faster than f32r)
       pb   = ones_blockdiag @ et  TensorE bf16 (per-half col-sum + bcast)
       ot   = min(et*recip(pb),1)*255 -> uint8   ONE custom DVE op
                                   (RECIP_MUL_ANT): bit-trick seed + 1
                                   Newton step + mul + clamp + scale,
                                   ±0.18% — replaces the old separate
                                   reciprocal + multiply (halves DVE work)
  * ALL x tiles are prefetched up front (64KB/partition fits SBUF), group
    0+1 singly and before the params so compute starts at first arrival.
  * uint8 fixed-point output (posterior is in [0,1]; quantum 0.004 vs the
    2e-2 budget) halves store traffic; host unstacks and rescales.
"""

import numpy as np
import ml_dtypes

import concourse.bacc as bacc
import concourse.bass as bass
import concourse.tile as tile
from concourse import mybir
from concourse.bass_utils import run_bass_kernel_spmd

B, D, N, K = 16, 128, 16384, 64
NCORES = 8
BPC = B // NCORES   # batches per core
PW = 1024           # columns per stacked half (2 PSUM banks)
GRP = 2 * PW        # n-columns consumed per group (halves A+B)

# per-group split of the 2048 square columns across the three elementwise
# engines, by measured HW rates (DVE fp16 2x ~0.6 ns/col, ScalarE ~1.24,
# Pool ~2.46) on top of each engine's fixed work (DVE: fused divide;
# ScalarE: exp)
SQ_POOL = 688
SQ_SCALAR = 480
SQ_DVE = GRP - SQ_POOL - SQ_SCALAR  # 880

F32 = mybir.dt.float32
F16 = mybir.dt.float16
BF16 = mybir.dt.bfloat16

# 1/x seed+NR constants shared with RECIPROCAL_APPROX_FAST (dve_ops.py)
RM_C0 = -0.23549792
RM_C1 = 2.0017324

_CACHE = {}


def _register_recip_mul():
    """Register the fused out = min(in1 * recip(in0), 1) * 255 custom DVE op.

    BITWISE_NOT exponent-flip seed + one inline Newton-Raphson pass
    (±0.18% rel err) + multiply by the second stream + clamp + uint8
    scale: 8 of 8 ALU stages, one DVE instruction per tile instead of
    reciprocal + multiply.  The clamp guards the uint8 write against the
    recip's +0.18% overshoot at posterior = 1."""
    import concourse.dve_ops as dve_ops

    for o in dve_ops.OPS:
        if o.name == "RECIP_MUL_ANT":
            return o
    from concourse.dve_spec import (
        Spec, Src0, Src1, C0, C1, C2, One, AluOp, Bin, lower, minn,
    )
    from concourse.dve_uop import DveOpSpec

    _not = Bin(AluOp.BITWISE_NOT, Src0, Src0)
    _y0 = _not * C0
    _y1 = _y0 * (C1 - Src0 * _y0)

    def _ref(in0, in1, s0, s1, imm2):
        not_x = (~in0.view(np.int32)).view(np.float32)
        y0 = not_x * s0
        y1 = y0 * (s1 - in0 * y0)
        return np.minimum(y1 * in1, 1.0) * imm2

    spec = Spec(body=minn(_y1 * Src1, One) * C2, reference=_ref)
    op = dve_ops.DveOp("RECIP_MUL_ANT", spec, subdim=False, uops_sha={})
    dve_ops.OPS.append(op)
    dve_ops.CUSTOM_DVE_SPECS[op.name] = spec
    dve_ops._SUB_OPCODE_FOR_NAME[op.name] = (
        dve_ops._CUSTOM_DVE_ROW_BASE + len(dve_ops.OPS) - 1
    )
    assert dve_ops._SUB_OPCODE_FOR_NAME[op.name] < 0x20
    for ver in ("v3", "v4"):
        ds = DveOpSpec(
            name=op.name,
            opcode=dve_ops.get_dve_sub_opcode(op.name),
            uops=lower(spec, ver=ver),
            rd1_en=dve_ops.has_src1(spec),
        )
        op.uops_sha[ver] = ds.sha(ver)
    return op


def _build_nc():
    recip_mul = _register_recip_mul()

    # Bacc (not raw Bass): its compile() pass legalizes Tile's multi-wait
    # instructions down to the 1-wait-per-instruction hardware limit.
    nc = bacc.Bacc("TRN2", target_bir_lowering=False, debug=False)
    x_in = nc.declare_dram_parameter("x", [BPC, D, N], F16, isOutput=False)
    w1_in = nc.declare_dram_parameter("w1", [D, K], F16, isOutput=False)
    w2_in = nc.declare_dram_parameter("w2", [D, K], F16, isOutput=False)
    cc_in = nc.declare_dram_parameter("cc", [2 * K, 1], F32, isOutput=False)
    ones_in = nc.declare_dram_parameter("ones_bd", [2 * K, 2 * K], BF16, isOutput=False)
    # output stays in the stacked layout as fixed-point uint8 (posterior is
    # in [0,1]; quantum 1/255 = 0.004 abs err vs the 2e-2 budget): halves
    # store HBM traffic vs fp16.  The host unstacks and rescales.
    U8 = mybir.dt.uint8
    out_ext = nc.declare_dram_parameter("out", [BPC, 2 * K, N // 2], U8, isOutput=True)

    with tile.TileContext(nc) as tc:
        with (
            tc.tile_pool(name="consts", bufs=1) as consts,
            tc.tile_pool(name="xp", bufs=9) as xp,
            tc.tile_pool(name="xqp", bufs=4) as xqp,
            tc.tile_pool(name="ep", bufs=4) as ep,
            tc.tile_pool(name="op", bufs=4) as op,
            tc.tile_pool(name="pd", bufs=2, space="PSUM") as pdp,
            tc.tile_pool(name="pb", bufs=2, space="PSUM") as pbp,
        ):
            n_grp = N // GRP  # 8 per batch row
            groups = [(b, g) for b in range(BPC) for g in range(n_grp)]
            NG = len(groups)
            st = [dict() for _ in range(NG)]

            def s0_load(i):
                # one DMA covers two groups (8KB per partition row), except
                # the first two groups which load singly so compute starts
                # as early as possible
                if i >= 2 and i % 2 == 1:
                    st[i]["xt"] = st[i - 1]["xt_big"][:, GRP:]
                    return
                b, g = groups[i]
                n0 = g * GRP
                w = GRP if i < 2 else 2 * GRP
                xt = xp.tile([D, w], F16, tag="xt")
                nc.sync.dma_start(out=xt, in_=x_in[b, :, n0 : n0 + w])
                st[i]["xt_big"] = xt
                st[i]["xt"] = xt[:, :GRP]

            # the DMA engines process transfers in dispatch order: group 0+1
            # x tiles go first so squares start ~immediately, then the small
            # constants (needed from the first dist GEMM on), then the rest
            s0_load(0)
            s0_load(1)
            w1_sb = consts.tile([D, K], F16)
            nc.sync.dma_start(out=w1_sb, in_=w1_in[:, :])
            w2_sb = consts.tile([D, K], F16)
            nc.sync.dma_start(out=w2_sb, in_=w2_in[:, :])
            cc_sb = consts.tile([2 * K, 1], F32)
            nc.sync.dma_start(out=cc_sb, in_=cc_in[:, :])
            ones_bd = consts.tile([2 * K, 2 * K], BF16)
            nc.sync.dma_start(out=ones_bd, in_=ones_in[:, :])
            for i in range(2, NG):
                s0_load(i)

            def s1_square(i):
                xt = st[i]["xt"]
                xsq = xqp.tile([D, GRP], F16, tag="xsq")
                # DVE slice first: dist chunk c0A depends only on it, so the
                # first GEMMs never wait on the slow Pool op
                c0 = SQ_DVE
                c1 = SQ_DVE + SQ_SCALAR
                nc.vector.tensor_mul(xsq[:, :c0], xt[:, :c0], xt[:, :c0])
                nc.scalar.activation(
                    out=xsq[:, c0:c1], in_=xt[:, c0:c1],
                    func=mybir.ActivationFunctionType.Square,
                )
                nc.gpsimd.tensor_mul(xsq[:, c1:], xt[:, c1:], xt[:, c1:])
                st[i]["xsq"] = xsq

            def s2_dist(i):
                xt, xsq = st[i]["xt"], st[i]["xsq"]
                # halves A (cols 0:PW) and B (cols PW:2PW) stacked on the
                # 128 PSUM partitions via PE column tiling.  Each PSUM
                # bank holds 512 fp32 columns -> 2 chunks per half.
                # A/B alternation puts consecutive MMs on different PE
                # column tiles so they overlap in the array.
                pd_t = pdp.tile([2 * K, PW], F32, tag="pd")
                for h in range(PW // 512):
                    sl = slice(h * 512, (h + 1) * 512)
                    for w_sb, src, start, stop in (
                        (w1_sb, xsq, True, False),
                        (w2_sb, xt, False, True),
                    ):
                        for half, p0 in ((0, 0), (1, K)):
                            msl = slice(
                                half * PW + h * 512, half * PW + (h + 1) * 512
                            )
                            nc.tensor.matmul(
                                pd_t[p0 : p0 + K, sl], w_sb[:, :], src[:, msl],
                                start=start, stop=stop,
                            )
                st[i]["pd"] = pd_t

            def s3_exp(i):
                pd_t = st[i].pop("pd")
                # bf16: holds e^±17 (fp16 would overflow) and streams the
                # ones-GEMM at 16-bit PE speed
                et = ep.tile([2 * K, PW], BF16, tag="et")
                nc.scalar.activation(
                    out=et, in_=pd_t,
                    func=mybir.ActivationFunctionType.Exp,
                    bias=cc_sb, scale=1.0,
                )
                st[i]["et"] = et
                st[i].pop("xt")
                st[i].pop("xsq")

            def s4_den(i):
                et = st[i]["et"]
                # block-diagonal ones: rows 0:64 sum half A, 64:128 sum
                # half B, each broadcast to its own partition range
                pb_t = pbp.tile([2 * K, PW], F32, tag="pb")
                for h in range(PW // 512):
                    sl = slice(h * 512, (h + 1) * 512)
                    nc.tensor.matmul(
                        pb_t[:, sl], ones_bd[:, :], et[:, sl],
                        start=True, stop=True,
                    )
                st[i]["pb"] = pb_t

            def s5_div_store(i):
                et, pb_t = st[i].pop("et"), st[i].pop("pb")
                b, g = groups[i]
                ot = op.tile([2 * K, PW], U8, tag="ot")
                nc.vector._custom_dve(
                    _CACHE["recip_mul"],
                    out=ot,
                    in0=pb_t,
                    in1=et,
                    s0=RM_C0,
                    s1=RM_C1,
                    imm2=255.0,
                )
                nc.sync.dma_start(
                    out=out_ext[b, :, g * PW : (g + 1) * PW], in_=ot
                )

            _CACHE["recip_mul"] = recip_mul

            stages = [
                s1_square, s2_dist, s3_exp,
                s4_den, s5_div_store,
            ]
            NS = len(stages)
            # downstream stages emitted first within each tick so no
            # engine's in-order queue blocks a later group's earlier stage
            for tick in range(NG + NS - 1):
                for k in reversed(range(NS)):
                    i = tick - k
                    if 0 <= i < NG:
                        stages[k](i)
    nc.compile()
    return nc


def _host_params(mu, log_sigma, log_alpha):
    mu64 = mu.astype(np.float64)
    mu_n = mu64 / np.maximum(
        np.linalg.norm(mu64, axis=1, keepdims=True), 1e-12
    )
    sinv = np.exp(-log_sigma.astype(np.float64))  # (K, D)
    a1 = -sinv                                    # coeff of x^2 in logits
    a2 = 2.0 * mu_n * sinv                        # coeff of x
    c = (
        -np.sum(mu_n * mu_n * sinv, axis=1)
        + log_alpha.astype(np.float64)
        - 0.5 * np.sum(log_sigma.astype(np.float64), axis=1)
    )
    # center across K: softmax is invariant to per-n shifts, and this keeps
    # the on-device logits within exp()'s comfortable range (~±17)
    a1c = a1 - a1.mean(axis=0, keepdims=True)
    a2c = a2 - a2.mean(axis=0, keepdims=True)
    ccv = (c - c.mean()).astype(np.float32)
    w1 = np.ascontiguousarray(a1c.T, dtype=np.float16)  # (D, K)
    w2 = np.ascontiguousarray(a2c.T, dtype=np.float16)  # (D, K)
    cc = np.concatenate([ccv, ccv]).reshape(2 * K, 1)
    return w1, w2, cc


def _in_maps(x, mu, log_sigma, log_alpha):
    x16 = np.ascontiguousarray(np.asarray(x), dtype=np.float16)
    w1, w2, cc = _host_params(
        np.asarray(mu), np.asarray(log_sigma), np.asarray(log_alpha)
    )
    ones_bd = np.zeros((2 * K, 2 * K), dtype=ml_dtypes.bfloat16)
    ones_bd[:K, :K] = 1
    ones_bd[K:, K:] = 1
    return [
        {
            "x": x16[i * BPC : (i + 1) * BPC],
            "w1": w1,
            "w2": w2,
            "cc": cc,
            "ones_bd": ones_bd,
        }
        for i in range(NCORES)
    ]


def kernel(x, mu, log_sigma, log_alpha):
    if "nc" not in _CACHE:
        _CACHE["nc"] = _build_nc()
    nc = _CACHE["nc"]
    in_maps = _in_maps(x, mu, log_sigma, log_alpha)
    res = run_bass_kernel_spmd(nc, in_maps, list(range(NCORES))).results
    out = np.concatenate(
        [np.asarray(res[i]["out"]) for i in range(NCORES)], axis=0
    )
    # unstack: dev[b, h*64+k, g*PW+c] = posterior[b, k, g*GRP + h*PW + c],
    # then rescale the uint8 fixed-point encoding back to [0, 1] floats
    n_grp = N // GRP
    out = (
        out.reshape(B, 2, K, n_grp, PW)
        .transpose(0, 2, 3, 1, 4)
        .reshape(B, K, N)
    )
    return out.astype(np.float32) * np.float32(1.0 / 255.0)


# revision 11
# speedup vs baseline: 1.1100x; 1.1100x over previous
"""Trainium2 Bass kernel for DiagonalGMMPosterior (vq_codebook).

Reference computation (per batch b, descriptor n, cluster k):
    dist[k,n]  = sum_d (x[d,n] - mu_n[k,d])^2 * exp(-log_sigma[k,d])
    logits     = -dist + log_alpha[k] - 0.5 * sum_d log_sigma[k,d]
    out[k,n]   = softmax_k(logits)

Device strategy (8 NeuronCores, data-parallel over the batch axis):
  * Host folds all (K,D) parameter math into two GEMM weight matrices and
    a per-cluster constant, then CENTERS them across K (softmax is
    invariant to per-n shifts) so logits stay within ~±17 — no per-n max.
  * x is pre-converted to fp16 on host: halves input HBM traffic.
  * Two 1024-column halves A/B of each 2048-column group are STACKED on
    the 128 PSUM partitions (PE column tiling): A's dist GEMMs write
    partitions 0:64, B's write 64:128.  Everything downstream processes
    both halves in one 128-partition pass.
  * Pipeline per group (PW=1024 stacked cols):
       xsq  = x*x                  split DVE (fp16 2x mode) / ScalarE / Pool
       pd   = W1^T x^2 + W2^T x    TensorE fp16, A/B col-tiles interleaved
       et   = exp(pd + cc) -> bf16 ScalarE (bf16 holds e^±17; lets the
                                   ones-GEMM stream 2.5x faster than f32r)
       pb   = ones_blockdiag @ et  TensorE bf16 (per-half col-sum + bcast)
       ot   = min(et*recip(pb),1)*255 -> uint8   ONE custom DVE op
                                   (RECIP_MUL_ANT): bit-trick seed + 1
                                   Newton step + mul + clamp + scale,
                                   ±0.18% — replaces the old separate
                                   reciprocal + multiply (halves DVE work)
  * ALL x tiles are prefetched up front (64KB/partition fits SBUF), group
    0+1 singly and before the params so compute starts at first arrival.
  * uint8 fixed-point output (posterior is in [0,1]; quantum 0.004 vs the
    2e-2 budget) halves store traffic; host unstacks and rescales.
"""

import numpy as np
import ml_dtypes

import concourse.bacc as bacc
import concourse.bass as bass
import concourse.tile as tile
from concourse import mybir
from concourse.bass_utils import run_bass_kernel_spmd

B, D, N, K = 16, 128, 16384, 64
NCORES = 8
BPC = B // NCORES   # batches per core
PW = 1024           # columns per stacked half (2 PSUM banks)
GRP = 2 * PW        # n-columns consumed per group (halves A+B)

# per-group split of the 2048 square columns across the three elementwise
# engines, by measured HW rates (DVE fp16 2x ~0.6 ns/col, ScalarE ~1.24,
# Pool ~2.46) on top of each engine's fixed work (DVE: fused divide;
# ScalarE: exp)
SQ_POOL = 688
SQ_SCALAR = 480
SQ_DVE = GRP - SQ_POOL - SQ_SCALAR  # 880

F32 = mybir.dt.float32
F16 = mybir.dt.float16
BF16 = mybir.dt.bfloat16

# 1/x seed+NR constants shared with RECIPROCAL_APPROX_FAST (dve_ops.py)
RM_C0 = -0.23549792
RM_C1 = 2.0017324

_CACHE = {}


def _register_recip_mul():
    """Register the fused out = min(in1 * recip(in0), 1) * 255 custom DVE op.

    BITWISE_NOT exponent-flip seed + one inline Newton-Raphson pass
    (±0.18% rel err) + multiply by the second stream + clamp + uint8
    scale: 8 of 8 ALU stages, one DVE instruction per tile instead of
    reciprocal + multiply.  The clamp guards the uint8 write against the
    recip's +0.18% overshoot at posterior = 1."""
    import concourse.dve_ops as dve_ops

    for o in dve_ops.OPS:
        if o.name == "RECIP_MUL_ANT":
            return o
    from concourse.dve_spec import (
        Spec, Src0, Src1, C0, C1, C2, One, AluOp, Bin, lower, minn,
    )
    from concourse.dve_uop import DveOpSpec

    _not = Bin(AluOp.BITWISE_NOT, Src0, Src0)
    _y0 = _not * C0
    _y1 = _y0 * (C1 - Src0 * _y0)

    def _ref(in0, in1, s0, s1, imm2):
        not_x = (~in0.view(np.int32)).view(np.float32)
        y0 = not_x * s0
        y1 = y0 * (s1 - in0 * y0)
        return np.minimum(y1 * in1, 1.0) * imm2

    spec = Spec(body=minn(_y1 * Src1, One) * C2, reference=_ref)
    op = dve_ops.DveOp("RECIP_MUL_ANT", spec, subdim=False, uops_sha={})
    dve_ops.OPS.append(op)
    dve_ops.CUSTOM_DVE_SPECS[op.name] = spec
    dve_ops._SUB_OPCODE_FOR_NAME[op.name] = (
        dve_ops._CUSTOM_DVE_ROW_BASE + len(dve_ops.OPS) - 1
    )
    assert dve_ops._SUB_OPCODE_FOR_NAME[op.name] < 0x20
    for ver in ("v3", "v4"):
        ds = DveOpSpec(
            name=op.name,
            opcode=dve_ops.get_dve_sub_opcode(op.name),
            uops=lower(spec, ver=ver),
            rd1_en=dve_ops.has_src1(spec),
        )
        op.uops_sha[ver] = ds.sha(ver)
    return op


def _build_nc():
    recip_mul = _register_recip_mul()

    # Bacc (not raw Bass): its compile() pass legalizes Tile's multi-wait
    # instructions down to the 1-wait-per-instruction hardware limit.
    nc = bacc.Bacc("TRN2", target_bir_lowering=False, debug=False)
    x_in = nc.declare_dram_parameter("x", [BPC, D, N], F16, isOutput=False)
    w1_in = nc.declare_dram_parameter("w1", [D, K], F16, isOutput=False)
    w2_in = nc.declare_dram_parameter("w2", [D, K], F16, isOutput=False)
    cc_in = nc.declare_dram_parameter("cc", [2 * K, 1], F32, isOutput=False)
    ones_in = nc.declare_dram_parameter("ones_bd", [2 * K, 2 * K], BF16, isOutput=False)
    # output stays in the stacked layout as fixed-point uint8 (posterior is
    # in [0,1]; quantum 1/255 = 0.004 abs err vs the 2e-2 budget): halves
    # store HBM traffic vs fp16.  The host unstacks and rescales.
    U8 = mybir.dt.uint8
    out_ext = nc.declare_dram_parameter("out", [BPC, 2 * K, N // 2], U8, isOutput=True)

    with tile.TileContext(nc) as tc:
        with (
            tc.tile_pool(name="consts", bufs=1) as consts,
            tc.tile_pool(name="xp", bufs=9) as xp,
            tc.tile_pool(name="xqp", bufs=4) as xqp,
            tc.tile_pool(name="ep", bufs=4) as ep,
            # one ot buffer per group (16KB total): stores drain behind the
            # x loads in the DMA queues, so a smaller pool would make div(i)
            # wait on store(i-bufs) completing -- serializing the whole
            # back half of the pipeline behind the load stream
            tc.tile_pool(name="op", bufs=16) as op,
            tc.tile_pool(name="pd", bufs=2, space="PSUM") as pdp,
            tc.tile_pool(name="pb", bufs=2, space="PSUM") as pbp,
        ):
            n_grp = N // GRP  # 8 per batch row
            groups = [(b, g) for b in range(BPC) for g in range(n_grp)]
            NG = len(groups)
            st = [dict() for _ in range(NG)]

            def s0_load(i):
                # one DMA covers two groups (8KB per partition row), except
                # the first two groups which load singly so compute starts
                # as early as possible
                if i >= 2 and i % 2 == 1:
                    st[i]["xt"] = st[i - 1]["xt_big"][:, GRP:]
                    return
                b, g = groups[i]
                n0 = g * GRP
                w = GRP if i < 2 else 2 * GRP
                xt = xp.tile([D, w], F16, tag="xt")
                nc.sync.dma_start(out=xt, in_=x_in[b, :, n0 : n0 + w])
                st[i]["xt_big"] = xt
                st[i]["xt"] = xt[:, :GRP]

            # the DMA engines process transfers in dispatch order: group 0+1
            # x tiles go first so squares start ~immediately, then the small
            # constants (needed from the first dist GEMM on), then the rest
            s0_load(0)
            s0_load(1)
            w1_sb = consts.tile([D, K], F16)
            nc.sync.dma_start(out=w1_sb, in_=w1_in[:, :])
            w2_sb = consts.tile([D, K], F16)
            nc.sync.dma_start(out=w2_sb, in_=w2_in[:, :])
            cc_sb = consts.tile([2 * K, 1], F32)
            nc.sync.dma_start(out=cc_sb, in_=cc_in[:, :])
            ones_bd = consts.tile([2 * K, 2 * K], BF16)
            nc.sync.dma_start(out=ones_bd, in_=ones_in[:, :])
            for i in range(2, NG):
                s0_load(i)

            def s1_square(i):
                xt = st[i]["xt"]
                xsq = xqp.tile([D, GRP], F16, tag="xsq")
                # DVE slice first: dist chunk c0A depends only on it, so the
                # first GEMMs never wait on the slow Pool op
                c0 = SQ_DVE
                c1 = SQ_DVE + SQ_SCALAR
                nc.vector.tensor_mul(xsq[:, :c0], xt[:, :c0], xt[:, :c0])
                nc.scalar.activation(
                    out=xsq[:, c0:c1], in_=xt[:, c0:c1],
                    func=mybir.ActivationFunctionType.Square,
                )
                nc.gpsimd.tensor_mul(xsq[:, c1:], xt[:, c1:], xt[:, c1:])
                st[i]["xsq"] = xsq

            def s2_dist(i):
                xt, xsq = st[i]["xt"], st[i]["xsq"]
                # halves A (cols 0:PW) and B (cols PW:2PW) stacked on the
                # 128 PSUM partitions via PE column tiling.  Each PSUM
                # bank holds 512 fp32 columns -> 2 chunks per half.
                # A/B alternation puts consecutive MMs on different PE
                # column tiles so they overlap in the array.
                pd_t = pdp.tile([2 * K, PW], F32, tag="pd")
                for h in range(PW // 512):
                    sl = slice(h * 512, (h + 1) * 512)
                    for w_sb, src, start, stop in (
                        (w1_sb, xsq, True, False),
                        (w2_sb, xt, False, True),
                    ):
                        for half, p0 in ((0, 0), (1, K)):
                            msl = slice(
                                half * PW + h * 512, half * PW + (h + 1) * 512
                            )
                            nc.tensor.matmul(
                                pd_t[p0 : p0 + K, sl], w_sb[:, :], src[:, msl],
                                start=start, stop=stop,
                            )
                st[i]["pd"] = pd_t

            def s3_exp(i):
                pd_t = st[i].pop("pd")
                # bf16: holds e^±17 (fp16 would overflow) and streams the
                # ones-GEMM at 16-bit PE speed
                et = ep.tile([2 * K, PW], BF16, tag="et")
                nc.scalar.activation(
                    out=et, in_=pd_t,
                    func=mybir.ActivationFunctionType.Exp,
                    bias=cc_sb, scale=1.0,
                )
                st[i]["et"] = et
                st[i].pop("xt")
                st[i].pop("xsq")

            def s4_den(i):
                et = st[i]["et"]
                # block-diagonal ones: rows 0:64 sum half A, 64:128 sum
                # half B, each broadcast to its own partition range
                pb_t = pbp.tile([2 * K, PW], F32, tag="pb")
                for h in range(PW // 512):
                    sl = slice(h * 512, (h + 1) * 512)
                    nc.tensor.matmul(
                        pb_t[:, sl], ones_bd[:, :], et[:, sl],
                        start=True, stop=True,
                    )
                st[i]["pb"] = pb_t

            def s5_div_store(i):
                et, pb_t = st[i].pop("et"), st[i].pop("pb")
                b, g = groups[i]
                ot = op.tile([2 * K, PW], U8, tag="ot")
                nc.vector._custom_dve(
                    _CACHE["recip_mul"],
                    out=ot,
                    in0=pb_t,
                    in1=et,
                    s0=RM_C0,
                    s1=RM_C1,
                    imm2=255.0,
                )
                nc.sync.dma_start(
                    out=out_ext[b, :, g * PW : (g + 1) * PW], in_=ot
                )

            _CACHE["recip_mul"] = recip_mul

            stages = [
                s1_square, s2_dist, s3_exp,
                s4_den, s5_div_store,
            ]
            NS = len(stages)
            # downstream stages emitted first within each tick so no
            # engine's in-order queue blocks a later group's earlier stage
            for tick in range(NG + NS - 1):
                for k in reversed(range(NS)):
                    i = tick - k
                    if 0 <= i < NG:
                        stages[k](i)
    nc.compile()
    return nc


def _host_params(mu, log_sigma, log_alpha):
    mu64 = mu.astype(np.float64)
    mu_n = mu64 / np.maximum(
        np.linalg.norm(mu64, axis=1, keepdims=True), 1e-12
    )
    sinv = np.exp(-log_sigma.astype(np.float64))  # (K, D)
    a1 = -sinv                                    # coeff of x^2 in logits
    a2 = 2.0 * mu_n * sinv                        # coeff of x
    c = (
        -np.sum(mu_n * mu_n * sinv, axis=1)
        + log_alpha.astype(np.float64)
        - 0.5 * np.sum(log_sigma.astype(np.float64), axis=1)
    )
    # center across K: softmax is invariant to per-n shifts, and this keeps
    # the on-device logits within exp()'s comfortable range (~±17)
    a1c = a1 - a1.mean(axis=0, keepdims=True)
    a2c = a2 - a2.mean(axis=0, keepdims=True)
    ccv = (c - c.mean()).astype(np.float32)
    w1 = np.ascontiguousarray(a1c.T, dtype=np.float16)  # (D, K)
    w2 = np.ascontiguousarray(a2c.T, dtype=np.float16)  # (D, K)
    cc = np.concatenate([ccv, ccv]).reshape(2 * K, 1)
    return w1, w2, cc


def _in_maps(x, mu, log_sigma, log_alpha):
    x16 = np.ascontiguousarray(np.asarray(x), dtype=np.float16)
    w1, w2, cc = _host_params(
        np.asarray(mu), np.asarray(log_sigma), np.asarray(log_alpha)
    )
    ones_bd = np.zeros((2 * K, 2 * K), dtype=ml_dtypes.bfloat16)
    ones_bd[:K, :K] = 1
    ones_bd[K:, K:] = 1
    return [
        {
            "x": x16[i * BPC : (i + 1) * BPC],
            "w1": w1,
            "w2": w2,
            "cc": cc,
            "ones_bd": ones_bd,
        }
        for i in range(NCORES)
    ]


def kernel(x, mu, log_sigma, log_alpha):
    if "nc" not in _CACHE:
        _CACHE["nc"] = _build_nc()
    nc = _CACHE["nc"]
    in_maps = _in_maps(x, mu, log_sigma, log_alpha)
    res = run_bass_kernel_spmd(nc, in_maps, list(range(NCORES))).results
    out = np.concatenate(
        [np.asarray(res[i]["out"]) for i in range(NCORES)], axis=0
    )
    # unstack: dev[b, h*64+k, g*PW+c] = posterior[b, k, g*GRP + h*PW + c],
    # then rescale the uint8 fixed-point encoding back to [0, 1] floats
    n_grp = N // GRP
    out = (
        out.reshape(B, 2, K, n_grp, PW)
        .transpose(0, 2, 3, 1, 4)
        .reshape(B, K, N)
    )
    return out.astype(np.float32) * np.float32(1.0 / 255.0)
